# revision 24
# baseline (speedup 1.0000x reference)
"""Trainium2 Bass kernel: causal multi-head attention with softmax over the
QUERY axis (faithful to the reference's softmax(dim=-2) quirk).

Problem shapes: B=2, T=2048, E=1024, H=16, D=64.

Sharding: 8 cores = 2 batches x 4 head-groups (4 heads per core, zero
communication).  Host pre-transposes X to (E, T) per batch and repacks it
(and the weights) into the exact SBUF partition-major layout so every input
DMA is a fully contiguous row copy.

v2 schedule (single continuous stream, ACT-paced):
  - phase B projects ONLY pair 0's Q_T/K_T (~14us) so the exp stream starts
    ~40us earlier than the all-pairs-first schedule.
  - one merged attention loop over 32 (pair, s-block) units; V projections
    and pair-1's Q/K projections are emitted as PE "feeder" jobs between
    early units, filling the PE while ACT chews the big exps.
  - causal mask added on DVE directly in PSUM on the diagonal block only.
  - exp on ACT with fused accum_out => denominators come free.
  - O_T per pair accumulated in a rotating 2-bank PSUM half with the
    deferred high-t AV trick (blocks 0-7's high-t contributions replayed
    once the second half opens).
"""

import numpy as np
from contextlib import ExitStack

B, T, E, H, D = 2, 2048, 1024, 16, 64
NCORES = 8
PAIRS = 2          # head pairs per core (4 heads)
EC = E // 128      # 8 contraction chunks
TB = T // 128      # 16 s-blocks
TC = T // 512      # 4 output column chunks
SCALE = float(D) ** -0.5

_CACHE = {}


def _st_chunks(h0, h1):
    """Split [h0, h1) into <=512 pieces aligned to the psum tile's own 512
    grid (tile column 0 is at absolute t=h0)."""
    out = []
    c0 = h0
    while c0 < h1:
        w = min(512, h1 - c0)
        out.append((c0, w))
        c0 += w
    return out


def _av_chunks(s0):
    """Split [s0, 2048) on the absolute 512 grid (psum_o bank alignment)."""
    out = []
    for j in range(s0 // 512, TC):
        c0 = max(s0, 512 * j)
        out.append((j, c0, 512 * (j + 1) - c0))
    return out


def _emit(tc, io):
    """Emit the kernel program into TileContext tc.  io: dict name -> AP."""
    import concourse.bass as bass
    import concourse.mybir as mybir

    nc = tc.nc
    fp32 = mybir.dt.float32
    fp16 = mybir.dt.float16
    AF = mybir.ActivationFunctionType
    ALU = mybir.AluOpType

    x_t, wq, wk, wv = io["x_t"], io["wq"], io["wk"], io["wv"]
    bq, bk, bv, out = io["bq"], io["bk"], io["bv"], io["out"]

    with ExitStack() as ctx:
        const = ctx.enter_context(tc.tile_pool(name="const", bufs=1))
        big = ctx.enter_context(tc.tile_pool(name="big", bufs=1))
        epool = ctx.enter_context(tc.tile_pool(name="epool", bufs=11))
        opool = ctx.enter_context(tc.tile_pool(name="opool", bufs=2))
        small = ctx.enter_context(tc.tile_pool(name="small", bufs=8))
        vpool = ctx.enter_context(tc.tile_pool(name="vpool", bufs=12))
        # "ps" slots are (128,1536) = 3 banks each, bufs=2 -> 6 banks, so a
        # score block up to 1536 wide needs ONE ACTIVATE (80 total vs 96).
        pp = ctx.enter_context(tc.tile_pool(name="pp", bufs=2, space="PSUM"))
        # O_T half accumulator: (128,1024) = 2 banks, single slot.
        po = ctx.enter_context(tc.tile_pool(name="po", bufs=1, space="PSUM"))

        # ---------- constants + inputs ----------
        # X_T and weights arrive as fp16, packed host-side as (128, EC*n)
        # partition-major rows so every DMA is one contiguous copy.
        xt_sb = big.tile([128, EC, T], fp16, tag="xt")
        xt_view = x_t.rearrange("p (c t) -> p c t", c=EC)

        wq_sb = const.tile([128, EC, 256], fp16, tag="wq")
        wk_sb = const.tile([128, EC, 256], fp16, tag="wk")
        wv_sb = const.tile([128, EC, 256], fp16, tag="wv")
        # wq/wk first (phase B gate), then X chunk-by-chunk on the scalar
        # HWDGE queue so the c-outer projection loop starts on chunk 0 while
        # later chunks stream; wv (first needed ~25us in) last.
        for w_sb_, w_ in ((wq_sb, wq), (wk_sb, wk)):
            nc.sync.dma_start(out=w_sb_,
                              in_=w_.rearrange("p (c n) -> p c n", c=EC))
        for c in range(EC):
            nc.scalar.dma_start(out=xt_sb[:, c, :], in_=xt_view[:, c, :])
        nc.sync.dma_start(out=wv_sb,
                          in_=wv.rearrange("p (c n) -> p c n", c=EC))

        bq_sb = const.tile([128, PAIRS], fp32, tag="bq")
        bk_sb = const.tile([128, PAIRS], fp32, tag="bk")
        for p in range(PAIRS):
            nc.gpsimd.dma_start(out=bq_sb[:, p:p + 1], in_=bq[p, :, None])
            nc.gpsimd.dma_start(out=bk_sb[:, p:p + 1], in_=bk[p, :, None])

        bv_sb = const.tile([128, 256], fp32, tag="bv")
        bv_bcast = bass.AP(tensor=bv.tensor, offset=bv.offset,
                           ap=[[0, 128]] + list(bv.ap))
        nc.gpsimd.dma_start(out=bv_sb, in_=bv_bcast)

        # PE warm-up: dummy matmuls during the input-DMA wait so the HAM
        # clock gate reaches 2.4 GHz before real work starts
        warm_sb = const.tile([128, 512], fp16, tag="warm")
        nc.vector.memset(warm_sb, 0.0)
        wps = po.tile([128, 1024], fp32, tag="po", name="ps_warm")
        for _ in range(8):
            nc.tensor.matmul(wps[:, 0:512], lhsT=warm_sb[:, 0:128],
                             rhs=warm_sb, start=True, stop=True)

        # mask[s, t] = 0 if t >= s else MNEG, added on DVE on the diag block
        MNEG = -60000.0
        mask_sb = const.tile([128, 128], fp16, tag="mask")
        nc.vector.memset(mask_sb, 0.0)
        nc.gpsimd.affine_select(
            out=mask_sb, in_=mask_sb,
            pattern=[[1, 128]], channel_multiplier=-1, base=0,
            compare_op=ALU.is_ge, fill=MNEG,
        )

        # ---------- phase B: pair-0 Q/K projections only ----------
        qt_sb = [big.tile([128, T], fp16, tag=f"qt{p}", name=f"qt{p}") for p in range(PAIRS)]
        kt_sb = [big.tile([128, T], fp16, tag=f"kt{p}", name=f"kt{p}") for p in range(PAIRS)]
        v_sb = big.tile([128, TB, 256], fp16, tag="v")

        # c-outermost over all 8 PSUM banks: the projection finishes ~2us
        # after the last X chunk lands instead of re-walking chunks twice.
        # The 8 (tensor, j) 512-wide regions are packed into the two 1536-
        # wide pp slots plus one 1024-wide po slot.
        scqA = pp.tile([128, 1536], fp32, tag="ps", name="scqA")
        scqB = pp.tile([128, 1536], fp32, tag="ps", name="scqB")
        sckh = po.tile([128, 1024], fp32, tag="po", name="sckh")
        bmap = {("q", 0): (scqA, 0), ("q", 1): (scqA, 512),
                ("q", 2): (scqB, 0), ("q", 3): (scqB, 512),
                ("k", 0): (scqA, 1024), ("k", 1): (scqB, 1024),
                ("k", 2): (sckh, 0), ("k", 3): (sckh, 512)}
        groups = (("q", qt_sb[0], bq_sb, wq_sb), ("k", kt_sb[0], bk_sb, wk_sb))
        for c in range(EC):
            last = c == EC - 1
            for tn, dst, b_sb, w_sb in groups:
                for j in range(4):
                    tile_, c0 = bmap[(tn, j)]
                    nc.tensor.matmul(
                        tile_[:, c0:c0 + 512],
                        lhsT=w_sb[:, c, 0:128],
                        rhs=xt_sb[:, c, 512 * j:512 * (j + 1)],
                        start=(c == 0), stop=last,
                    )
                    if last:
                        # bias-add j overlaps the j+1 matmul's stream
                        nc.vector.tensor_scalar_add(
                            out=dst[:, 512 * j:512 * (j + 1)],
                            in0=tile_[:, c0:c0 + 512],
                            scalar1=b_sb[:, 0:1],
                        )

        # ---------- feeder jobs (run inside the attention stream) ----------
        vstate = {"next": 0}

        def emit_v_blocks(tbs):
            for tb in tbs:
                ps = pp.tile([128, 1536], fp32, tag="ps", name="ps_v")
                for c in range(EC):
                    nc.tensor.matmul(
                        ps[:, 0:256],
                        lhsT=xt_sb[:, c, 128 * tb:128 * (tb + 1)],
                        rhs=wv_sb[:, c, :],
                        start=(c == 0), stop=(c == EC - 1),
                    )
                nc.vector.tensor_tensor(out=v_sb[:, tb, :], in0=ps[:, 0:256],
                                        in1=bv_sb, op=ALU.add)

        def emit_v_upto(tb_needed):
            while vstate["next"] <= min(tb_needed, TB - 1):
                emit_v_blocks([vstate["next"]])
                vstate["next"] += 1

        def emit_qk1_job(jc):
            """Project pair 1's Q and K for t-chunk jc (one PSUM slot)."""
            sc = pp.tile([128, 1536], fp32, tag="ps", name=f"qk1_{jc}")
            for c in range(EC):
                nc.tensor.matmul(
                    sc[:, 0:512], lhsT=wq_sb[:, c, 128:256],
                    rhs=xt_sb[:, c, 512 * jc:512 * (jc + 1)],
                    start=(c == 0), stop=(c == EC - 1),
                )
                nc.tensor.matmul(
                    sc[:, 512:1024], lhsT=wk_sb[:, c, 128:256],
                    rhs=xt_sb[:, c, 512 * jc:512 * (jc + 1)],
                    start=(c == 0), stop=(c == EC - 1),
                )
            nc.vector.tensor_scalar_add(
                out=qt_sb[1][:, 512 * jc:512 * (jc + 1)],
                in0=sc[:, 0:512], scalar1=bq_sb[:, 1:2])
            nc.vector.tensor_scalar_add(
                out=kt_sb[1][:, 512 * jc:512 * (jc + 1)],
                in0=sc[:, 512:1024], scalar1=bk_sb[:, 1:2])

        # ---------- attention: one merged stream over 32 (pair, block) ----
        P = {}
        for p in range(PAIRS):
            P[p] = {"po_half": {}, "saved": {}, "o_sb": None,
                    "defer": None, "navs": 0, "flushed_hi": False}
        from collections import deque
        for p in range(PAIRS):
            P[p]["defer"] = deque()
            P[p]["o_sb"] = opool.tile([128, T], fp32, tag=f"o{p}",
                                      name=f"o_sb{p}")

        def open_half(p, ph):
            t = po.tile([128, 1024], fp32, tag="po", name=f"po_{p}_{ph}")
            P[p]["po_half"][ph] = t

        def emit_st_exp(p, i):
            s0 = 128 * i
            halves = [(h0, min(T, h0 + 1536))
                      for h0 in (s0, s0 + 1536) if h0 < T]
            e_t = [epool.tile([128, T], fp16, tag=f"e{h}", name=f"e{h}")
                   for h in range(2)]
            den = small.tile([128, 2, 2], fp32, tag="den", name="den")
            pss = {}
            for hf, (h0, h1) in enumerate(halves):
                for h in range(2):
                    pss[(h, hf)] = pp.tile([128, 1536], fp32, tag="ps",
                                           name="ps_st")
                for c0, cw in _st_chunks(h0, h1):
                    diag = hf == 0 and c0 == h0
                    # 4 quadrant MMs: (head h) x (s sub-block q); the q pair
                    # shares one rhs stream and runs concurrently
                    for h in range(2):
                        for q in range(2):
                            nc.tensor.matmul(
                                pss[(h, hf)][64 * q:64 * (q + 1),
                                             c0 - h0:c0 - h0 + cw],
                                lhsT=kt_sb[p][64 * h:64 * (h + 1),
                                              s0 + 64 * q:s0 + 64 * (q + 1)],
                                rhs=qt_sb[p][64 * h:64 * (h + 1), c0:c0 + cw],
                                start=True, stop=True,
                                tile_position=(64 * h, 64 * q),
                                skip_group_check=True,
                            )
                    if diag:
                        # causal mask add (DVE, on PSUM; PE stream stays clean)
                        for h in range(2):
                            nc.vector.tensor_tensor(
                                out=pss[(h, hf)][:, 0:128],
                                in0=pss[(h, hf)][:, 0:128],
                                in1=mask_sb, op=ALU.add)
                for h in range(2):
                    wh = h1 - h0
                    nc.scalar.activation(
                        out=e_t[h][:, h0 - s0:h0 - s0 + wh],
                        in_=pss[(h, hf)][:, 0:wh], func=AF.Exp,
                        scale=SCALE, accum_out=den[:, h, hf:hf + 1],
                    )
            return i, len(halves), e_t, den

        def emit_norm_av(p, st):
            i, nhalves, e_t, den = st
            if 0 not in P[p]["po_half"]:
                open_half(p, 0)
            rec = small.tile([128, 2], fp32, tag="rec", name="rec")
            if nhalves == 2:
                nc.vector.tensor_reduce(
                    out=rec, in_=den[:, :, :],
                    axis=mybir.AxisListType.X, op=ALU.add)
                nc.vector.reciprocal(rec, rec)
            else:
                nc.vector.reciprocal(rec, den[:, :, 0])
            vp = vpool.tile([128, 128], fp16, tag="vp", name="vp")
            for h in range(2):
                nc.vector.tensor_scalar_mul(
                    out=vp[:, 64 * h:64 * (h + 1)],
                    in0=v_sb[:, i, 128 * p + 64 * h:128 * p + 64 * (h + 1)],
                    scalar1=rec[:, h:h + 1],
                )
            emit_av(p, i, e_t, vp, (0, 1) if i <= 7 else (2, 3))
            if i <= 7:
                P[p]["saved"][i] = (e_t, vp)
            if i == 3:
                flush_chunk(p, 0)
            if i == 7:
                flush_chunk(p, 1)
                open_half(p, 1)
            P[p]["navs"] += 1

        def emit_av(p, i, e_t, vp, jlist):
            s0 = 128 * i
            for j, c0, cw in _av_chunks(s0):
                if j not in jlist:
                    continue
                ph = 0 if j < 2 else 1
                base = 1024 * ph
                for h in range(2):
                    nc.tensor.matmul(
                        P[p]["po_half"][ph][64 * h:64 * (h + 1),
                                            c0 - base:c0 - base + cw],
                        lhsT=vp[:, 64 * h:64 * (h + 1)],
                        rhs=e_t[h][:, c0 - s0:c0 - s0 + cw],
                        start=(i == 0), stop=False,
                        tile_position=(0, 64 * h),
                        skip_group_check=True,
                    )

        def flush_chunk(p, j):
            ph = 0 if j < 2 else 1
            base = 1024 * ph
            o_sb = P[p]["o_sb"]
            nc.vector.tensor_copy(
                o_sb[:, 512 * j:512 * (j + 1)],
                P[p]["po_half"][ph][:, 512 * j - base:512 * (j + 1) - base])
            nc.sync.dma_start(out=out[p][:, 512 * j:512 * (j + 1)],
                              in_=o_sb[:, 512 * j:512 * (j + 1)])

        def drain_one_defer():
            for pp_ in range(PAIRS):
                st = P[pp_]
                n = 2 if pp_ == 1 else 1
                while n and 1 in st["po_half"] and st["defer"]:
                    k = st["defer"].popleft()
                    e_t, vp = st["saved"][k]
                    emit_av(pp_, k, e_t, vp, (2, 3))
                    n -= 1
                if n < (2 if pp_ == 1 else 1):
                    return

        def finish_pair(p):
            """Flush high half once all AV for pair p has been emitted."""
            st = P[p]
            if (not st["flushed_hi"] and st["navs"] == TB
                    and not st["defer"]):
                flush_chunk(p, 2)
                flush_chunk(p, 3)
                st["flushed_hi"] = True

        units = [(0, i) for i in range(TB)] + [(1, i) for i in range(TB)]
        qk1_at = {1: 0, 3: 1, 5: 2, 7: 3}
        pend = deque()
        for (p, i) in units:
            pend.append((p, emit_st_exp(p, i)))
            if len(pend) > 2:
                pp_, st = pend.popleft()
                emit_norm_av(pp_, st)
                if st[0] <= 7:
                    P[pp_]["defer"].append(st[0])
            if p == 0:
                emit_v_upto(i)
                if i in qk1_at:
                    emit_qk1_job(qk1_at[i])
            drain_one_defer()
            finish_pair(0)
        while pend:
            pp_, st = pend.popleft()
            emit_norm_av(pp_, st)
            if st[0] <= 7:
                P[pp_]["defer"].append(st[0])
        for p in range(PAIRS):
            while P[p]["defer"]:
                drain_one_defer()
            finish_pair(p)


def _build():
    """Build + schedule + compile the (SPMD-identical) program once."""
    if "nc" in _CACHE:
        return _CACHE["nc"]
    import concourse.bacc as bacc
    import concourse.mybir as mybir
    import concourse.tile as tile

    fp32 = mybir.dt.float32
    fp16 = mybir.dt.float16
    nc = bacc.Bacc("TRN2", target_bir_lowering=False, debug=False)
    io = {
        "x_t": nc.dram_tensor("x_t", [128, EC * T], fp16, kind="ExternalInput").ap(),
        "wq": nc.dram_tensor("wq", [128, EC * 256], fp16, kind="ExternalInput").ap(),
        "wk": nc.dram_tensor("wk", [128, EC * 256], fp16, kind="ExternalInput").ap(),
        "wv": nc.dram_tensor("wv", [128, EC * 256], fp16, kind="ExternalInput").ap(),
        "bq": nc.dram_tensor("bq", [PAIRS, 128], fp32, kind="ExternalInput").ap(),
        "bk": nc.dram_tensor("bk", [PAIRS, 128], fp32, kind="ExternalInput").ap(),
        "bv": nc.dram_tensor("bv", [256], fp32, kind="ExternalInput").ap(),
        "out": nc.dram_tensor("out", [PAIRS, 128, T], fp32,
                              kind="ExternalOutput").ap(),
    }
    with tile.TileContext(nc) as tc:
        _emit(tc, io)
    nc.compile()
    _CACHE["nc"] = nc
    return nc


def _pack_rows(a):
    """(E, n) -> (128, EC*n) partition-major contiguous fp16 rows."""
    n = a.shape[1]
    return np.ascontiguousarray(
        a.astype(np.float16)
        .reshape(EC, 128, n).transpose(1, 0, 2).reshape(128, EC * n))


def make_in_maps(inputs_x, k_w, k_b, q_w, q_b, v_w, v_b):
    """Host-side sharding: per-core input dict."""
    xs = [_pack_rows(np.ascontiguousarray(inputs_x[b].T))
          for b in range(B)]
    in_maps = []
    for core in range(NCORES):
        b, g = divmod(core, 4)
        hs = range(4 * g, 4 * g + 4)
        pack_w = lambda w: _pack_rows(
            np.concatenate([w[h] for h in hs], axis=1))
        pack_b2 = lambda bb: np.ascontiguousarray(
            bb[4 * g:4 * g + 4].reshape(PAIRS, 128).astype(np.float32, copy=False))
        in_maps.append({
            "x_t": xs[b],
            "wq": pack_w(q_w), "wk": pack_w(k_w), "wv": pack_w(v_w),
            "bq": pack_b2(q_b), "bk": pack_b2(k_b),
            "bv": np.ascontiguousarray(
                v_b[4 * g:4 * g + 4].reshape(256).astype(np.float32, copy=False)),
        })
    return in_maps


def assemble(core_outs):
    """Gather per-core (PAIRS, 128, T) outputs into the full (B, T, H*D)."""
    out_full = np.empty((B, T, H * D), np.float32)
    for core in range(NCORES):
        b, g = divmod(core, 4)
        o = core_outs[core]
        for p in range(PAIRS):
            out_full[b, :, g * 256 + 128 * p:g * 256 + 128 * (p + 1)] = o[p].T
    return out_full


def kernel(**inputs):
    x = np.asarray(inputs["inputs"], np.float32)
    args = [np.asarray(inputs[k], np.float32)
            for k in ("k_w", "k_b", "q_w", "q_b", "v_w", "v_b")]
    from concourse.bass_utils import run_bass_kernel_spmd

    nc = _build()
    in_maps = make_in_maps(x, *args)
    res = run_bass_kernel_spmd(nc, in_maps, core_ids=list(range(NCORES)))
    return assemble([r["out"] for r in res.results])


# revision 26
# speedup vs baseline: 1.3586x; 1.3586x over previous
"""Trainium2 Bass kernel: causal multi-head attention with softmax over the
QUERY axis (faithful to the reference's softmax(dim=-2) quirk).

Problem shapes: B=2, T=2048, E=1024, H=16, D=64.

Sharding: 8 cores = 2 batches x 4 head-groups (4 heads per core, zero
communication).  Host pre-transposes X to (E, T) per batch and repacks it
(and the weights) into the exact SBUF partition-major layout so every input
DMA is a fully contiguous row copy.

v2 schedule (single continuous stream, ACT-paced):
  - phase B projects ONLY pair 0's Q_T/K_T (~14us) so the exp stream starts
    ~40us earlier than the all-pairs-first schedule.
  - one merged attention loop over 32 (pair, s-block) units; V projections
    and pair-1's Q/K projections are emitted as PE "feeder" jobs between
    early units, filling the PE while ACT chews the big exps.
  - causal mask added on DVE directly in PSUM on the diagonal block only.
  - exp on ACT with fused accum_out => denominators come free.
  - O_T per pair accumulated in a rotating 2-bank PSUM half with the
    deferred high-t AV trick (blocks 0-7's high-t contributions replayed
    once the second half opens).
"""

import numpy as np
from contextlib import ExitStack

B, T, E, H, D = 2, 2048, 1024, 16, 64
NCORES = 8
PAIRS = 2          # head pairs per core (4 heads)
EC = E // 128      # 8 contraction chunks
TB = T // 128      # 16 s-blocks
TC = T // 512      # 4 output column chunks
SCALE = float(D) ** -0.5

_CACHE = {}


def _st_chunks(h0, h1):
    """Split [h0, h1) into <=512 pieces aligned to the psum tile's own 512
    grid (tile column 0 is at absolute t=h0)."""
    out = []
    c0 = h0
    while c0 < h1:
        w = min(512, h1 - c0)
        out.append((c0, w))
        c0 += w
    return out


def _av_chunks(s0):
    """Split [s0, 2048) on the absolute 512 grid (psum_o bank alignment)."""
    out = []
    for j in range(s0 // 512, TC):
        c0 = max(s0, 512 * j)
        out.append((j, c0, 512 * (j + 1) - c0))
    return out


def _emit(tc, io):
    """Emit the kernel program into TileContext tc.  io: dict name -> AP."""
    import concourse.bass as bass
    import concourse.mybir as mybir

    nc = tc.nc
    fp32 = mybir.dt.float32
    fp16 = mybir.dt.float16
    AF = mybir.ActivationFunctionType
    ALU = mybir.AluOpType

    x_t, wq, wk, wv = io["x_t"], io["wq"], io["wk"], io["wv"]
    bq, bk, bv, out = io["bq"], io["bk"], io["bv"], io["out"]

    with ExitStack() as ctx:
        const = ctx.enter_context(tc.tile_pool(name="const", bufs=1))
        big = ctx.enter_context(tc.tile_pool(name="big", bufs=1))
        epool = ctx.enter_context(tc.tile_pool(name="epool", bufs=11))
        opool = ctx.enter_context(tc.tile_pool(name="opool", bufs=2))
        small = ctx.enter_context(tc.tile_pool(name="small", bufs=8))
        vpool = ctx.enter_context(tc.tile_pool(name="vpool", bufs=12))
        # "ps" slots are (128,1024) = 2 banks each, bufs=3 -> 6 banks.
        pp = ctx.enter_context(tc.tile_pool(name="pp", bufs=3, space="PSUM"))
        # O_T half accumulator: (128,1024) = 2 banks, single slot.
        po = ctx.enter_context(tc.tile_pool(name="po", bufs=1, space="PSUM"))

        # ---------- constants + inputs ----------
        # X_T and weights arrive as fp16, packed host-side as (128, EC*n)
        # partition-major rows so every DMA is one contiguous copy.
        xt_sb = big.tile([128, EC, T], fp16, tag="xt")
        xt_view = x_t.rearrange("p (c t) -> p c t", c=EC)

        wq_sb = const.tile([128, EC, 256], fp16, tag="wq")
        wk_sb = const.tile([128, EC, 256], fp16, tag="wk")
        wv_sb = const.tile([128, EC, 256], fp16, tag="wv")
        # wq/wk first (phase B gate), then X chunk-by-chunk on the scalar
        # HWDGE queue so the c-outer projection loop starts on chunk 0 while
        # later chunks stream; wv (first needed ~25us in) last.
        for w_sb_, w_ in ((wq_sb, wq), (wk_sb, wk)):
            nc.sync.dma_start(out=w_sb_,
                              in_=w_.rearrange("p (c n) -> p c n", c=EC))
        for c in range(EC):
            nc.scalar.dma_start(out=xt_sb[:, c, :], in_=xt_view[:, c, :])
        nc.sync.dma_start(out=wv_sb,
                          in_=wv.rearrange("p (c n) -> p c n", c=EC))

        bq_sb = const.tile([128, PAIRS], fp32, tag="bq")
        bk_sb = const.tile([128, PAIRS], fp32, tag="bk")
        for p in range(PAIRS):
            nc.gpsimd.dma_start(out=bq_sb[:, p:p + 1], in_=bq[p, :, None])
            nc.gpsimd.dma_start(out=bk_sb[:, p:p + 1], in_=bk[p, :, None])

        bv_sb = const.tile([128, 256], fp32, tag="bv")
        bv_bcast = bass.AP(tensor=bv.tensor, offset=bv.offset,
                           ap=[[0, 128]] + list(bv.ap))
        nc.gpsimd.dma_start(out=bv_sb, in_=bv_bcast)

        # PE warm-up: dummy matmuls during the input-DMA wait so the HAM
        # clock gate reaches 2.4 GHz before real work starts
        warm_sb = const.tile([128, 512], fp16, tag="warm")
        nc.vector.memset(warm_sb, 0.0)
        wps = po.tile([128, 1024], fp32, tag="po", name="ps_warm")
        for _ in range(8):
            nc.tensor.matmul(wps[:, 0:512], lhsT=warm_sb[:, 0:128],
                             rhs=warm_sb, start=True, stop=True)

        # mask[s, t] = 0 if t >= s else MNEG, applied on PE as identity@mask
        # (a DVE mask-add head-of-line blocks the ACT queue behind feeder
        # bias-adds; the PE path keeps the exp stream self-contained)
        MNEG = -60000.0
        mask_sb = const.tile([128, 128], fp16, tag="mask")
        nc.vector.memset(mask_sb, 0.0)
        nc.gpsimd.affine_select(
            out=mask_sb, in_=mask_sb,
            pattern=[[1, 128]], channel_multiplier=-1, base=0,
            compare_op=ALU.is_ge, fill=MNEG,
        )
        ident_sb = const.tile([128, 128], fp16, tag="ident")
        nc.vector.memset(ident_sb, 0.0)
        nc.gpsimd.affine_select(
            out=ident_sb, in_=ident_sb,
            pattern=[[1, 128]], channel_multiplier=-1, base=0,
            compare_op=ALU.not_equal, fill=1.0,
        )

        # ---------- phase B: pair-0 Q/K projections only ----------
        qt_sb = [big.tile([128, T], fp16, tag=f"qt{p}", name=f"qt{p}") for p in range(PAIRS)]
        kt_sb = [big.tile([128, T], fp16, tag=f"kt{p}", name=f"kt{p}") for p in range(PAIRS)]
        v_sb = big.tile([128, TB, 256], fp16, tag="v")

        # c-outermost over all 8 PSUM banks: the projection finishes ~2us
        # after the last X chunk lands instead of re-walking chunks twice.
        scq = [pp.tile([128, 1024], fp32, tag="ps", name="scq0"),
               pp.tile([128, 1024], fp32, tag="ps", name="scq1")]
        sck = [pp.tile([128, 1024], fp32, tag="ps", name="sck0"),
               po.tile([128, 1024], fp32, tag="po", name="sck1")]
        groups = ((scq, qt_sb[0], bq_sb, wq_sb), (sck, kt_sb[0], bk_sb, wk_sb))
        for c in range(EC):
            last = c == EC - 1
            for scs, dst, b_sb, w_sb in groups:
                for j in range(4):
                    nc.tensor.matmul(
                        scs[j // 2][:, 512 * (j % 2):512 * (j % 2 + 1)],
                        lhsT=w_sb[:, c, 0:128],
                        rhs=xt_sb[:, c, 512 * j:512 * (j + 1)],
                        start=(c == 0), stop=last,
                    )
                    if last:
                        # bias-add j overlaps the j+1 matmul's stream
                        nc.vector.tensor_scalar_add(
                            out=dst[:, 512 * j:512 * (j + 1)],
                            in0=scs[j // 2][:, 512 * (j % 2):512 * (j % 2 + 1)],
                            scalar1=b_sb[:, 0:1],
                        )

        # ---------- feeder jobs (run inside the attention stream) ----------
        vstate = {"next": 0}

        def emit_v_blocks(tbs):
            for tb in tbs:
                ps = pp.tile([128, 1024], fp32, tag="ps", name="ps_v")
                for c in range(EC):
                    nc.tensor.matmul(
                        ps[:, 0:256],
                        lhsT=xt_sb[:, c, 128 * tb:128 * (tb + 1)],
                        rhs=wv_sb[:, c, :],
                        start=(c == 0), stop=(c == EC - 1),
                    )
                nc.vector.tensor_tensor(out=v_sb[:, tb, :], in0=ps[:, 0:256],
                                        in1=bv_sb, op=ALU.add)

        def emit_v_upto(tb_needed):
            while vstate["next"] <= min(tb_needed, TB - 1):
                emit_v_blocks([vstate["next"]])
                vstate["next"] += 1

        def emit_qk1_job(jc):
            """Project pair 1's Q and K for t-chunk jc (one PSUM slot)."""
            sc = pp.tile([128, 1024], fp32, tag="ps", name=f"qk1_{jc}")
            for c in range(EC):
                nc.tensor.matmul(
                    sc[:, 0:512], lhsT=wq_sb[:, c, 128:256],
                    rhs=xt_sb[:, c, 512 * jc:512 * (jc + 1)],
                    start=(c == 0), stop=(c == EC - 1),
                )
                nc.tensor.matmul(
                    sc[:, 512:1024], lhsT=wk_sb[:, c, 128:256],
                    rhs=xt_sb[:, c, 512 * jc:512 * (jc + 1)],
                    start=(c == 0), stop=(c == EC - 1),
                )
            nc.vector.tensor_scalar_add(
                out=qt_sb[1][:, 512 * jc:512 * (jc + 1)],
                in0=sc[:, 0:512], scalar1=bq_sb[:, 1:2])
            nc.vector.tensor_scalar_add(
                out=kt_sb[1][:, 512 * jc:512 * (jc + 1)],
                in0=sc[:, 512:1024], scalar1=bk_sb[:, 1:2])

        # ---------- attention: one merged stream over 32 (pair, block) ----
        P = {}
        for p in range(PAIRS):
            P[p] = {"po_half": {}, "saved": {}, "o_sb": None,
                    "defer": None, "navs": 0, "flushed_hi": False}
        from collections import deque
        for p in range(PAIRS):
            P[p]["defer"] = deque()
            P[p]["o_sb"] = opool.tile([128, T], fp32, tag=f"o{p}",
                                      name=f"o_sb{p}")

        def open_half(p, ph):
            t = po.tile([128, 1024], fp32, tag="po", name=f"po_{p}_{ph}")
            P[p]["po_half"][ph] = t

        def emit_st_exp(p, i):
            s0 = 128 * i
            halves = [(h0, min(T, h0 + 1024))
                      for h0 in (s0, s0 + 1024) if h0 < T]
            e_t = [epool.tile([128, T], fp16, tag=f"e{h}", name=f"e{h}")
                   for h in range(2)]
            den = small.tile([128, 2, 2], fp32, tag="den", name="den")
            pss = {}
            for hf, (h0, h1) in enumerate(halves):
                for h in range(2):
                    pss[(h, hf)] = pp.tile([128, 1024], fp32, tag="ps",
                                           name="ps_st")
                for c0, cw in _st_chunks(h0, h1):
                    diag = hf == 0 and c0 == h0
                    # 4 quadrant MMs: (head h) x (s sub-block q); the q pair
                    # shares one rhs stream and runs concurrently
                    for h in range(2):
                        for q in range(2):
                            nc.tensor.matmul(
                                pss[(h, hf)][64 * q:64 * (q + 1),
                                             c0 - h0:c0 - h0 + cw],
                                lhsT=kt_sb[p][64 * h:64 * (h + 1),
                                              s0 + 64 * q:s0 + 64 * (q + 1)],
                                rhs=qt_sb[p][64 * h:64 * (h + 1), c0:c0 + cw],
                                start=True, stop=True,
                                tile_position=(64 * h, 64 * q),
                                skip_group_check=True,
                            )
                    if diag:
                        # causal mask add on PE: += identity.T @ mask
                        for h in range(2):
                            nc.tensor.matmul(
                                pss[(h, hf)][:, 0:128],
                                lhsT=ident_sb, rhs=mask_sb,
                                start=False, stop=True,
                                skip_group_check=True,
                            )
                for h in range(2):
                    wh = h1 - h0
                    nc.scalar.activation(
                        out=e_t[h][:, h0 - s0:h0 - s0 + wh],
                        in_=pss[(h, hf)][:, 0:wh], func=AF.Exp,
                        scale=SCALE, accum_out=den[:, h, hf:hf + 1],
                    )
            return i, len(halves), e_t, den

        def emit_norm_av(p, st):
            i, nhalves, e_t, den = st
            if 0 not in P[p]["po_half"]:
                open_half(p, 0)
            rec = small.tile([128, 2], fp32, tag="rec", name="rec")
            if nhalves == 2:
                nc.vector.tensor_reduce(
                    out=rec, in_=den[:, :, :],
                    axis=mybir.AxisListType.X, op=ALU.add)
                nc.vector.reciprocal(rec, rec)
            else:
                nc.vector.reciprocal(rec, den[:, :, 0])
            vp = vpool.tile([128, 128], fp16, tag="vp", name="vp")
            for h in range(2):
                nc.vector.tensor_scalar_mul(
                    out=vp[:, 64 * h:64 * (h + 1)],
                    in0=v_sb[:, i, 128 * p + 64 * h:128 * p + 64 * (h + 1)],
                    scalar1=rec[:, h:h + 1],
                )
            emit_av(p, i, e_t, vp, (0, 1) if i <= 7 else (2, 3))
            if i <= 7:
                P[p]["saved"][i] = (e_t, vp)
            if i == 3:
                flush_chunk(p, 0)
            if i == 7:
                flush_chunk(p, 1)
                open_half(p, 1)
            P[p]["navs"] += 1

        def emit_av(p, i, e_t, vp, jlist):
            s0 = 128 * i
            for j, c0, cw in _av_chunks(s0):
                if j not in jlist:
                    continue
                ph = 0 if j < 2 else 1
                base = 1024 * ph
                for h in range(2):
                    nc.tensor.matmul(
                        P[p]["po_half"][ph][64 * h:64 * (h + 1),
                                            c0 - base:c0 - base + cw],
                        lhsT=vp[:, 64 * h:64 * (h + 1)],
                        rhs=e_t[h][:, c0 - s0:c0 - s0 + cw],
                        start=(i == 0), stop=False,
                        tile_position=(0, 64 * h),
                        skip_group_check=True,
                    )

        def flush_chunk(p, j):
            ph = 0 if j < 2 else 1
            base = 1024 * ph
            o_sb = P[p]["o_sb"]
            nc.vector.tensor_copy(
                o_sb[:, 512 * j:512 * (j + 1)],
                P[p]["po_half"][ph][:, 512 * j - base:512 * (j + 1) - base])
            nc.sync.dma_start(out=out[p][:, 512 * j:512 * (j + 1)],
                              in_=o_sb[:, 512 * j:512 * (j + 1)])

        def drain_one_defer():
            for pp_ in range(PAIRS):
                st = P[pp_]
                n = 2 if pp_ == 1 else 1
                while n and 1 in st["po_half"] and st["defer"]:
                    k = st["defer"].popleft()
                    e_t, vp = st["saved"][k]
                    emit_av(pp_, k, e_t, vp, (2, 3))
                    n -= 1
                if n < (2 if pp_ == 1 else 1):
                    return

        def finish_pair(p):
            """Flush high half once all AV for pair p has been emitted."""
            st = P[p]
            if (not st["flushed_hi"] and st["navs"] == TB
                    and not st["defer"]):
                flush_chunk(p, 2)
                flush_chunk(p, 3)
                st["flushed_hi"] = True

        units = [(0, i) for i in range(TB)] + [(1, i) for i in range(TB)]
        qk1_at = {1: 0, 3: 1, 5: 2, 7: 3}
        pend = deque()
        for (p, i) in units:
            pend.append((p, emit_st_exp(p, i)))
            if len(pend) > 2:
                pp_, st = pend.popleft()
                emit_norm_av(pp_, st)
                if st[0] <= 7:
                    P[pp_]["defer"].append(st[0])
            if p == 0:
                emit_v_upto(i)
                if i in qk1_at:
                    emit_qk1_job(qk1_at[i])
            drain_one_defer()
            finish_pair(0)
        while pend:
            pp_, st = pend.popleft()
            emit_norm_av(pp_, st)
            if st[0] <= 7:
                P[pp_]["defer"].append(st[0])
        for p in range(PAIRS):
            while P[p]["defer"]:
                drain_one_defer()
            finish_pair(p)


def _build():
    """Build + schedule + compile the (SPMD-identical) program once."""
    if "nc" in _CACHE:
        return _CACHE["nc"]
    import concourse.bacc as bacc
    import concourse.mybir as mybir
    import concourse.tile as tile

    fp32 = mybir.dt.float32
    fp16 = mybir.dt.float16
    nc = bacc.Bacc("TRN2", target_bir_lowering=False, debug=False)
    io = {
        "x_t": nc.dram_tensor("x_t", [128, EC * T], fp16, kind="ExternalInput").ap(),
        "wq": nc.dram_tensor("wq", [128, EC * 256], fp16, kind="ExternalInput").ap(),
        "wk": nc.dram_tensor("wk", [128, EC * 256], fp16, kind="ExternalInput").ap(),
        "wv": nc.dram_tensor("wv", [128, EC * 256], fp16, kind="ExternalInput").ap(),
        "bq": nc.dram_tensor("bq", [PAIRS, 128], fp32, kind="ExternalInput").ap(),
        "bk": nc.dram_tensor("bk", [PAIRS, 128], fp32, kind="ExternalInput").ap(),
        "bv": nc.dram_tensor("bv", [256], fp32, kind="ExternalInput").ap(),
        "out": nc.dram_tensor("out", [PAIRS, 128, T], fp32,
                              kind="ExternalOutput").ap(),
    }
    with tile.TileContext(nc) as tc:
        _emit(tc, io)
    nc.compile()
    _CACHE["nc"] = nc
    return nc


def _pack_rows(a):
    """(E, n) -> (128, EC*n) partition-major contiguous fp16 rows."""
    n = a.shape[1]
    return np.ascontiguousarray(
        a.astype(np.float16)
        .reshape(EC, 128, n).transpose(1, 0, 2).reshape(128, EC * n))


def make_in_maps(inputs_x, k_w, k_b, q_w, q_b, v_w, v_b):
    """Host-side sharding: per-core input dict."""
    xs = [_pack_rows(np.ascontiguousarray(inputs_x[b].T))
          for b in range(B)]
    in_maps = []
    for core in range(NCORES):
        b, g = divmod(core, 4)
        hs = range(4 * g, 4 * g + 4)
        pack_w = lambda w: _pack_rows(
            np.concatenate([w[h] for h in hs], axis=1))
        pack_b2 = lambda bb: np.ascontiguousarray(
            bb[4 * g:4 * g + 4].reshape(PAIRS, 128).astype(np.float32, copy=False))
        in_maps.append({
            "x_t": xs[b],
            "wq": pack_w(q_w), "wk": pack_w(k_w), "wv": pack_w(v_w),
            "bq": pack_b2(q_b), "bk": pack_b2(k_b),
            "bv": np.ascontiguousarray(
                v_b[4 * g:4 * g + 4].reshape(256).astype(np.float32, copy=False)),
        })
    return in_maps


def assemble(core_outs):
    """Gather per-core (PAIRS, 128, T) outputs into the full (B, T, H*D)."""
    out_full = np.empty((B, T, H * D), np.float32)
    for core in range(NCORES):
        b, g = divmod(core, 4)
        o = core_outs[core]
        for p in range(PAIRS):
            out_full[b, :, g * 256 + 128 * p:g * 256 + 128 * (p + 1)] = o[p].T
    return out_full


def kernel(**inputs):
    x = np.asarray(inputs["inputs"], np.float32)
    args = [np.asarray(inputs[k], np.float32)
            for k in ("k_w", "k_b", "q_w", "q_b", "v_w", "v_b")]
    from concourse.bass_utils import run_bass_kernel_spmd

    nc = _build()
    in_maps = make_in_maps(x, *args)
    res = run_bass_kernel_spmd(nc, in_maps, core_ids=list(range(NCORES)))
    return assemble([r["out"] for r in res.results])


# revision 27
# speedup vs baseline: 1.4108x; 1.0384x over previous
"""Trainium2 Bass kernel: causal multi-head attention with softmax over the
QUERY axis (faithful to the reference's softmax(dim=-2) quirk).

Problem shapes: B=2, T=2048, E=1024, H=16, D=64.

Sharding: 8 cores = 2 batches x 4 head-groups (4 heads per core, zero
communication).  Host pre-transposes X to (E, T) per batch and repacks it
(and the weights) into the exact SBUF partition-major layout so every input
DMA is a fully contiguous row copy.

v2 schedule (single continuous stream, ACT-paced):
  - phase B projects ONLY pair 0's Q_T/K_T (~14us) so the exp stream starts
    ~40us earlier than the all-pairs-first schedule.
  - one merged attention loop over 32 (pair, s-block) units; V projections
    and pair-1's Q/K projections are emitted as PE "feeder" jobs between
    early units, filling the PE while ACT chews the big exps.
  - causal mask added on DVE directly in PSUM on the diagonal block only.
  - exp on ACT with fused accum_out => denominators come free.
  - O_T per pair accumulated in a rotating 2-bank PSUM half with the
    deferred high-t AV trick (blocks 0-7's high-t contributions replayed
    once the second half opens).
"""

import numpy as np
from contextlib import ExitStack

B, T, E, H, D = 2, 2048, 1024, 16, 64
NCORES = 8
PAIRS = 2          # head pairs per core (4 heads)
EC = E // 128      # 8 contraction chunks
TB = T // 128      # 16 s-blocks
TC = T // 512      # 4 output column chunks
SCALE = float(D) ** -0.5

_CACHE = {}


def _st_chunks(h0, h1):
    """Split [h0, h1) into <=512 pieces aligned to the psum tile's own 512
    grid (tile column 0 is at absolute t=h0)."""
    out = []
    c0 = h0
    while c0 < h1:
        w = min(512, h1 - c0)
        out.append((c0, w))
        c0 += w
    return out


def _av_chunks(s0):
    """Split [s0, 2048) on the absolute 512 grid (psum_o bank alignment)."""
    out = []
    for j in range(s0 // 512, TC):
        c0 = max(s0, 512 * j)
        out.append((j, c0, 512 * (j + 1) - c0))
    return out


def _emit(tc, io):
    """Emit the kernel program into TileContext tc.  io: dict name -> AP."""
    import concourse.bass as bass
    import concourse.mybir as mybir

    nc = tc.nc
    fp32 = mybir.dt.float32
    fp16 = mybir.dt.float16
    AF = mybir.ActivationFunctionType
    ALU = mybir.AluOpType

    x_t, wq, wk, wv = io["x_t"], io["wq"], io["wk"], io["wv"]
    bq, bk, bv, out = io["bq"], io["bk"], io["bv"], io["out"]

    with ExitStack() as ctx:
        const = ctx.enter_context(tc.tile_pool(name="const", bufs=1))
        big = ctx.enter_context(tc.tile_pool(name="big", bufs=1))
        epool = ctx.enter_context(tc.tile_pool(name="epool", bufs=11))
        opool = ctx.enter_context(tc.tile_pool(name="opool", bufs=2))
        small = ctx.enter_context(tc.tile_pool(name="small", bufs=24))
        vpool = ctx.enter_context(tc.tile_pool(name="vpool", bufs=18))
        # "ps" slots are (128,1024) = 2 banks each, bufs=3 -> 6 banks.
        pp = ctx.enter_context(tc.tile_pool(name="pp", bufs=3, space="PSUM"))
        # O_T half accumulator: (128,1024) = 2 banks, single slot.
        po = ctx.enter_context(tc.tile_pool(name="po", bufs=1, space="PSUM"))

        # ---------- constants + inputs ----------
        # X_T and weights arrive as fp16, packed host-side as (128, EC*n)
        # partition-major rows so every DMA is one contiguous copy.
        xt_sb = big.tile([128, EC, T], fp16, tag="xt")
        xt_view = x_t.rearrange("p (c t) -> p c t", c=EC)

        wq_sb = const.tile([128, EC, 256], fp16, tag="wq")
        wk_sb = const.tile([128, EC, 256], fp16, tag="wk")
        wv_sb = const.tile([128, EC, 256], fp16, tag="wv")
        # wq/wk first (phase B gate), then X chunk-by-chunk on the scalar
        # HWDGE queue so the c-outer projection loop starts on chunk 0 while
        # later chunks stream; wv (first needed ~25us in) last.
        for w_sb_, w_ in ((wq_sb, wq), (wk_sb, wk)):
            nc.sync.dma_start(out=w_sb_,
                              in_=w_.rearrange("p (c n) -> p c n", c=EC))
        for c in range(EC):
            nc.scalar.dma_start(out=xt_sb[:, c, :], in_=xt_view[:, c, :])
        nc.sync.dma_start(out=wv_sb,
                          in_=wv.rearrange("p (c n) -> p c n", c=EC))

        bq_sb = const.tile([128, PAIRS], fp32, tag="bq")
        bk_sb = const.tile([128, PAIRS], fp32, tag="bk")
        for p in range(PAIRS):
            nc.gpsimd.dma_start(out=bq_sb[:, p:p + 1], in_=bq[p, :, None])
            nc.gpsimd.dma_start(out=bk_sb[:, p:p + 1], in_=bk[p, :, None])

        bv_sb = const.tile([128, 256], fp32, tag="bv")
        bv_bcast = bass.AP(tensor=bv.tensor, offset=bv.offset,
                           ap=[[0, 128]] + list(bv.ap))
        nc.gpsimd.dma_start(out=bv_sb, in_=bv_bcast)

        # PE warm-up: dummy matmuls during the input-DMA wait so the HAM
        # clock gate reaches 2.4 GHz before real work starts
        warm_sb = const.tile([128, 512], fp16, tag="warm")
        nc.vector.memset(warm_sb, 0.0)
        wps = po.tile([128, 1024], fp32, tag="po", name="ps_warm")
        for _ in range(8):
            nc.tensor.matmul(wps[:, 0:512], lhsT=warm_sb[:, 0:128],
                             rhs=warm_sb, start=True, stop=True)

        # mask[s, t] = 0 if t >= s else MNEG, applied on PE as identity@mask
        # (a DVE mask-add head-of-line blocks the ACT queue behind feeder
        # bias-adds; the PE path keeps the exp stream self-contained)
        MNEG = -60000.0
        mask_sb = const.tile([128, 128], fp16, tag="mask")
        nc.vector.memset(mask_sb, 0.0)
        nc.gpsimd.affine_select(
            out=mask_sb, in_=mask_sb,
            pattern=[[1, 128]], channel_multiplier=-1, base=0,
            compare_op=ALU.is_ge, fill=MNEG,
        )
        ident_sb = const.tile([128, 128], fp16, tag="ident")
        nc.vector.memset(ident_sb, 0.0)
        nc.gpsimd.affine_select(
            out=ident_sb, in_=ident_sb,
            pattern=[[1, 128]], channel_multiplier=-1, base=0,
            compare_op=ALU.not_equal, fill=1.0,
        )

        # ---------- phase B: pair-0 Q/K projections only ----------
        qt_sb = [big.tile([128, T], fp16, tag=f"qt{p}", name=f"qt{p}") for p in range(PAIRS)]
        kt_sb = [big.tile([128, T], fp16, tag=f"kt{p}", name=f"kt{p}") for p in range(PAIRS)]
        v_sb = big.tile([128, TB, 256], fp16, tag="v")

        # c-outermost over all 8 PSUM banks: the projection finishes ~2us
        # after the last X chunk lands instead of re-walking chunks twice.
        scq = [pp.tile([128, 1024], fp32, tag="ps", name="scq0"),
               pp.tile([128, 1024], fp32, tag="ps", name="scq1")]
        sck = [pp.tile([128, 1024], fp32, tag="ps", name="sck0"),
               po.tile([128, 1024], fp32, tag="po", name="sck1")]
        groups = ((scq, qt_sb[0], bq_sb, wq_sb), (sck, kt_sb[0], bk_sb, wk_sb))
        for c in range(EC):
            last = c == EC - 1
            for scs, dst, b_sb, w_sb in groups:
                for j in range(4):
                    nc.tensor.matmul(
                        scs[j // 2][:, 512 * (j % 2):512 * (j % 2 + 1)],
                        lhsT=w_sb[:, c, 0:128],
                        rhs=xt_sb[:, c, 512 * j:512 * (j + 1)],
                        start=(c == 0), stop=last,
                    )
                    if last:
                        # bias-add j overlaps the j+1 matmul's stream
                        nc.vector.tensor_scalar_add(
                            out=dst[:, 512 * j:512 * (j + 1)],
                            in0=scs[j // 2][:, 512 * (j % 2):512 * (j % 2 + 1)],
                            scalar1=b_sb[:, 0:1],
                        )

        # ---------- feeder jobs (run inside the attention stream) ----------
        vstate = {"next": 0}

        def emit_v_blocks(tbs):
            for tb in tbs:
                ps = pp.tile([128, 1024], fp32, tag="ps", name="ps_v")
                for c in range(EC):
                    nc.tensor.matmul(
                        ps[:, 0:256],
                        lhsT=xt_sb[:, c, 128 * tb:128 * (tb + 1)],
                        rhs=wv_sb[:, c, :],
                        start=(c == 0), stop=(c == EC - 1),
                    )
                nc.vector.tensor_tensor(out=v_sb[:, tb, :], in0=ps[:, 0:256],
                                        in1=bv_sb, op=ALU.add)

        def emit_v_upto(tb_needed):
            while vstate["next"] <= min(tb_needed, TB - 1):
                emit_v_blocks([vstate["next"]])
                vstate["next"] += 1

        def emit_qk1_job(jc):
            """Project pair 1's Q and K for t-chunk jc (one PSUM slot)."""
            sc = pp.tile([128, 1024], fp32, tag="ps", name=f"qk1_{jc}")
            for c in range(EC):
                nc.tensor.matmul(
                    sc[:, 0:512], lhsT=wq_sb[:, c, 128:256],
                    rhs=xt_sb[:, c, 512 * jc:512 * (jc + 1)],
                    start=(c == 0), stop=(c == EC - 1),
                )
                nc.tensor.matmul(
                    sc[:, 512:1024], lhsT=wk_sb[:, c, 128:256],
                    rhs=xt_sb[:, c, 512 * jc:512 * (jc + 1)],
                    start=(c == 0), stop=(c == EC - 1),
                )
            nc.vector.tensor_scalar_add(
                out=qt_sb[1][:, 512 * jc:512 * (jc + 1)],
                in0=sc[:, 0:512], scalar1=bq_sb[:, 1:2])
            nc.vector.tensor_scalar_add(
                out=kt_sb[1][:, 512 * jc:512 * (jc + 1)],
                in0=sc[:, 512:1024], scalar1=bk_sb[:, 1:2])

        # ---------- attention: one merged stream over 32 (pair, block) ----
        P = {}
        for p in range(PAIRS):
            P[p] = {"po_half": {}, "saved": {}, "o_sb": None,
                    "defer": None, "navs": 0, "flushed_hi": False}
        from collections import deque
        for p in range(PAIRS):
            P[p]["defer"] = deque()
            P[p]["o_sb"] = opool.tile([128, T], fp32, tag=f"o{p}",
                                      name=f"o_sb{p}")

        def open_half(p, ph):
            t = po.tile([128, 1024], fp32, tag="po", name=f"po_{p}_{ph}")
            P[p]["po_half"][ph] = t

        def emit_st_exp(p, i):
            s0 = 128 * i
            halves = [(h0, min(T, h0 + 1024))
                      for h0 in (s0, s0 + 1024) if h0 < T]
            e_t = [epool.tile([128, T], fp16, tag=f"e{h}", name=f"e{h}")
                   for h in range(2)]
            den = small.tile([128, 2, 2], fp32, tag="den", name="den")
            pss = {}
            for hf, (h0, h1) in enumerate(halves):
                for h in range(2):
                    pss[(h, hf)] = pp.tile([128, 1024], fp32, tag="ps",
                                           name="ps_st")
                for c0, cw in _st_chunks(h0, h1):
                    diag = hf == 0 and c0 == h0
                    # 4 quadrant MMs: (head h) x (s sub-block q); the q pair
                    # shares one rhs stream and runs concurrently
                    for h in range(2):
                        for q in range(2):
                            nc.tensor.matmul(
                                pss[(h, hf)][64 * q:64 * (q + 1),
                                             c0 - h0:c0 - h0 + cw],
                                lhsT=kt_sb[p][64 * h:64 * (h + 1),
                                              s0 + 64 * q:s0 + 64 * (q + 1)],
                                rhs=qt_sb[p][64 * h:64 * (h + 1), c0:c0 + cw],
                                start=True, stop=True,
                                tile_position=(64 * h, 64 * q),
                                skip_group_check=True,
                            )
                    if diag:
                        # causal mask add on PE: += identity.T @ mask
                        for h in range(2):
                            nc.tensor.matmul(
                                pss[(h, hf)][:, 0:128],
                                lhsT=ident_sb, rhs=mask_sb,
                                start=False, stop=True,
                                skip_group_check=True,
                            )
                for h in range(2):
                    wh = h1 - h0
                    nc.scalar.activation(
                        out=e_t[h][:, h0 - s0:h0 - s0 + wh],
                        in_=pss[(h, hf)][:, 0:wh], func=AF.Exp,
                        scale=SCALE, accum_out=den[:, h, hf:hf + 1],
                    )
            return i, len(halves), e_t, den

        def emit_norm_av(p, st):
            i, nhalves, e_t, den = st
            if 0 not in P[p]["po_half"]:
                open_half(p, 0)
            rec = small.tile([128, 2], fp32, tag="rec", name="rec")
            if nhalves == 2:
                nc.vector.tensor_reduce(
                    out=rec, in_=den[:, :, :],
                    axis=mybir.AxisListType.X, op=ALU.add)
                nc.vector.reciprocal(rec, rec)
            else:
                nc.vector.reciprocal(rec, den[:, :, 0])
            vp = vpool.tile([128, 128], fp16, tag="vp", name="vp")
            for h in range(2):
                nc.vector.tensor_scalar_mul(
                    out=vp[:, 64 * h:64 * (h + 1)],
                    in0=v_sb[:, i, 128 * p + 64 * h:128 * p + 64 * (h + 1)],
                    scalar1=rec[:, h:h + 1],
                )
            emit_av(p, i, e_t, vp, (0, 1) if i <= 7 else (2, 3))
            if i <= 7:
                P[p]["saved"][i] = (e_t, vp)
            if i == 3:
                flush_chunk(p, 0)
            if i == 7:
                flush_chunk(p, 1)
                open_half(p, 1)
            P[p]["navs"] += 1

        def emit_av(p, i, e_t, vp, jlist):
            s0 = 128 * i
            for j, c0, cw in _av_chunks(s0):
                if j not in jlist:
                    continue
                ph = 0 if j < 2 else 1
                base = 1024 * ph
                for h in range(2):
                    nc.tensor.matmul(
                        P[p]["po_half"][ph][64 * h:64 * (h + 1),
                                            c0 - base:c0 - base + cw],
                        lhsT=vp[:, 64 * h:64 * (h + 1)],
                        rhs=e_t[h][:, c0 - s0:c0 - s0 + cw],
                        start=(i == 0), stop=False,
                        tile_position=(0, 64 * h),
                        skip_group_check=True,
                    )

        def flush_chunk(p, j):
            ph = 0 if j < 2 else 1
            base = 1024 * ph
            o_sb = P[p]["o_sb"]
            nc.vector.tensor_copy(
                o_sb[:, 512 * j:512 * (j + 1)],
                P[p]["po_half"][ph][:, 512 * j - base:512 * (j + 1) - base])
            nc.sync.dma_start(out=out[p][:, 512 * j:512 * (j + 1)],
                              in_=o_sb[:, 512 * j:512 * (j + 1)])

        def drain_one_defer():
            for pp_ in range(PAIRS):
                st = P[pp_]
                n = 2 if pp_ == 1 else 1
                while n and 1 in st["po_half"] and st["defer"]:
                    k = st["defer"].popleft()
                    e_t, vp = st["saved"][k]
                    emit_av(pp_, k, e_t, vp, (2, 3))
                    n -= 1
                if n < (2 if pp_ == 1 else 1):
                    return

        def finish_pair(p):
            """Flush high half once all AV for pair p has been emitted."""
            st = P[p]
            if (not st["flushed_hi"] and st["navs"] == TB
                    and not st["defer"]):
                flush_chunk(p, 2)
                flush_chunk(p, 3)
                st["flushed_hi"] = True

        units = [(0, i) for i in range(TB)] + [(1, i) for i in range(TB)]
        qk1_at = {1: 0, 3: 1, 5: 2, 7: 3}
        pend = deque()
        for (p, i) in units:
            pend.append((p, emit_st_exp(p, i)))
            if len(pend) > 2:
                pp_, st = pend.popleft()
                emit_norm_av(pp_, st)
                if st[0] <= 7:
                    P[pp_]["defer"].append(st[0])
            if p == 0:
                emit_v_upto(i)
                if i in qk1_at:
                    emit_qk1_job(qk1_at[i])
            drain_one_defer()
            finish_pair(0)
        while pend:
            pp_, st = pend.popleft()
            emit_norm_av(pp_, st)
            if st[0] <= 7:
                P[pp_]["defer"].append(st[0])
        for p in range(PAIRS):
            while P[p]["defer"]:
                drain_one_defer()
            finish_pair(p)


def _build():
    """Build + schedule + compile the (SPMD-identical) program once."""
    if "nc" in _CACHE:
        return _CACHE["nc"]
    import concourse.bacc as bacc
    import concourse.mybir as mybir
    import concourse.tile as tile

    fp32 = mybir.dt.float32
    fp16 = mybir.dt.float16
    nc = bacc.Bacc("TRN2", target_bir_lowering=False, debug=False)
    io = {
        "x_t": nc.dram_tensor("x_t", [128, EC * T], fp16, kind="ExternalInput").ap(),
        "wq": nc.dram_tensor("wq", [128, EC * 256], fp16, kind="ExternalInput").ap(),
        "wk": nc.dram_tensor("wk", [128, EC * 256], fp16, kind="ExternalInput").ap(),
        "wv": nc.dram_tensor("wv", [128, EC * 256], fp16, kind="ExternalInput").ap(),
        "bq": nc.dram_tensor("bq", [PAIRS, 128], fp32, kind="ExternalInput").ap(),
        "bk": nc.dram_tensor("bk", [PAIRS, 128], fp32, kind="ExternalInput").ap(),
        "bv": nc.dram_tensor("bv", [256], fp32, kind="ExternalInput").ap(),
        "out": nc.dram_tensor("out", [PAIRS, 128, T], fp32,
                              kind="ExternalOutput").ap(),
    }
    with tile.TileContext(nc) as tc:
        _emit(tc, io)
    nc.compile()
    _CACHE["nc"] = nc
    return nc


def _pack_rows(a):
    """(E, n) -> (128, EC*n) partition-major contiguous fp16 rows."""
    n = a.shape[1]
    return np.ascontiguousarray(
        a.astype(np.float16)
        .reshape(EC, 128, n).transpose(1, 0, 2).reshape(128, EC * n))


def make_in_maps(inputs_x, k_w, k_b, q_w, q_b, v_w, v_b):
    """Host-side sharding: per-core input dict."""
    xs = [_pack_rows(np.ascontiguousarray(inputs_x[b].T))
          for b in range(B)]
    in_maps = []
    for core in range(NCORES):
        b, g = divmod(core, 4)
        hs = range(4 * g, 4 * g + 4)
        pack_w = lambda w: _pack_rows(
            np.concatenate([w[h] for h in hs], axis=1))
        pack_b2 = lambda bb: np.ascontiguousarray(
            bb[4 * g:4 * g + 4].reshape(PAIRS, 128).astype(np.float32, copy=False))
        in_maps.append({
            "x_t": xs[b],
            "wq": pack_w(q_w), "wk": pack_w(k_w), "wv": pack_w(v_w),
            "bq": pack_b2(q_b), "bk": pack_b2(k_b),
            "bv": np.ascontiguousarray(
                v_b[4 * g:4 * g + 4].reshape(256).astype(np.float32, copy=False)),
        })
    return in_maps


def assemble(core_outs):
    """Gather per-core (PAIRS, 128, T) outputs into the full (B, T, H*D)."""
    out_full = np.empty((B, T, H * D), np.float32)
    for core in range(NCORES):
        b, g = divmod(core, 4)
        o = core_outs[core]
        for p in range(PAIRS):
            out_full[b, :, g * 256 + 128 * p:g * 256 + 128 * (p + 1)] = o[p].T
    return out_full


def kernel(**inputs):
    x = np.asarray(inputs["inputs"], np.float32)
    args = [np.asarray(inputs[k], np.float32)
            for k in ("k_w", "k_b", "q_w", "q_b", "v_w", "v_b")]
    from concourse.bass_utils import run_bass_kernel_spmd

    nc = _build()
    in_maps = make_in_maps(x, *args)
    res = run_bass_kernel_spmd(nc, in_maps, core_ids=list(range(NCORES)))
    return assemble([r["out"] for r in res.results])
